# revision 8
# baseline (speedup 1.0000x reference)
"""EquiConv (DeepH-E3) Trainium2 kernel — 8-core data-parallel over edges.

v2 strategy (channel-major, fp16, engine-rebalanced):
  - Input per tile: 6 host blocks [x1s | x1va | x2d | fwt | vv | v2s]
    (vv=[v0;v1], v2s=[v2;s] are the only host-replicated x2 rows) plus an
    on-chip broadcast DMA that fans the compact x2 rows [s|v0|v1|v2] to
    all 128 partitions of the same SBUF tile (saves ~40% input HBM).
  - ACT does 4 ops/tile: fused silu of [h1(k); h2(k-1)] packed into one
    PSUM bank, silu(scal), tanh(gate/2), and the wwb+bias crossing.
  - DVE does the big fused prescales (fp16 2x mode), tga, the
    (wwa+b2a)*silu stt, and one fused out012 multiply over a 2-bank
    PSUM tile [vec01 | vec2].
  - Pool (gpsimd) takes two SBUF-only multiplies: xsp2 prescale and the
    sigmoid-gate product sgw2.
  - One output DMA per tile: O = [out_s | out01 | out2(+64 garbage rows)]
    as [128, 1536] fp16, unpacked on host.

Self-contained: hardcodes shapes from the problem spec; no file reads.
"""
import os
import sys

import numpy as np

# ---------------------------------------------------------------- constants
E_FULL = 200000
N_CORES = 8
E_CORE = E_FULL // N_CORES      # 25000
NT = 512                        # edges per tile
T_TILES = 49                    # tiles per core
E_PAD = NT * T_TILES            # 25088
MUL_S = 128
MUL_V = 64
N_BLK = 6                       # host input row-blocks per tile
TILE_COLS = N_BLK * NT          # 3072
SB_COLS = 10 * NT               # 5120 (6 host blocks + 4 bcast blocks)
OUT_BLK = 3 * NT                # 1536

INV_S = 1.0 / np.sqrt(MUL_S)
INV_V = 1.0 / np.sqrt(MUL_V)
SQ2 = 1.0 / np.sqrt(2.0)
SQ3 = 1.0 / np.sqrt(3.0)

_REPO_CANDIDATES = (
    "/opt/trn_rl_repo",
    "/root/.axon_site/_ro/trn_rl_repo",
)


def _ensure_repo_on_path():
    try:
        import concourse.bass  # noqa: F401
        return
    except ImportError:
        pass
    for p in _REPO_CANDIDATES:
        if os.path.isdir(p) and p not in sys.path:
            sys.path.insert(0, p)
    import concourse.bass  # noqa: F401


_CACHE = {}


def _build_nc():
    """Build + compile the per-core Bass program (cached)."""
    if "nc" in _CACHE:
        return _CACHE["nc"]
    _ensure_repo_on_path()
    import concourse.mybir as mybir
    import concourse.tile as tile
    from concourse import bacc

    F32 = mybir.dt.float32
    F16 = mybir.dt.float16
    MULT = mybir.AluOpType.mult
    ADD = mybir.AluOpType.add
    AF = mybir.ActivationFunctionType

    nc = bacc.Bacc(trn_type="TRN2", target_bir_lowering=False, debug=False,
                   num_devices=N_CORES)

    # DRAM inputs (per-core shard) ----------------------------------------
    d_in = nc.dram_tensor("in_t", [128, T_TILES * TILE_COLS], F16,
                          kind="ExternalInput")
    d_x2c = nc.dram_tensor("x2c", [4, E_PAD], F16, kind="ExternalInput")
    d_wa0 = nc.dram_tensor("wa0", [128, 128], F16, kind="ExternalInput")
    d_wa1d = nc.dram_tensor("wa1d", [128, 128], F16, kind="ExternalInput")
    d_w2lo = nc.dram_tensor("w2lo", [128, 128], F16, kind="ExternalInput")
    d_w2hi = nc.dram_tensor("w2hi", [128, 128], F16, kind="ExternalInput")
    d_w2v2 = nc.dram_tensor("w2v2", [128, 64], F16, kind="ExternalInput")
    d_wb4s = nc.dram_tensor("wb4s", [128, 128], F16, kind="ExternalInput")
    d_wb5sd = nc.dram_tensor("wb5sd", [128, 128], F16, kind="ExternalInput")
    d_wb4b = nc.dram_tensor("wb4b", [64, 128], F16, kind="ExternalInput")
    d_wb5bd = nc.dram_tensor("wb5bd", [64, 128], F16, kind="ExternalInput")
    d_wcdiag = nc.dram_tensor("wcdiag", [128, 128], F16, kind="ExternalInput")
    d_wc2 = nc.dram_tensor("wc2", [64, 64], F16, kind="ExternalInput")
    d_fc0 = nc.dram_tensor("fc0", [128, 64], F16, kind="ExternalInput")
    d_fc1 = nc.dram_tensor("fc1", [64, 64], F16, kind="ExternalInput")
    d_fc2a = nc.dram_tensor("fc2a", [64, 128], F16, kind="ExternalInput")
    d_fc2bd = nc.dram_tensor("fc2bd", [64, 128], F16, kind="ExternalInput")
    d_b01 = nc.dram_tensor("b01c", [128, 1], F32, kind="ExternalInput")
    d_b2a = nc.dram_tensor("b2a", [128, 1], F32, kind="ExternalInput")
    d_b2bh = nc.dram_tensor("b2bh", [128, 1], F32, kind="ExternalInput")

    d_out = nc.dram_tensor("out_t", [128, T_TILES * OUT_BLK], F16,
                           kind="ExternalOutput")

    with tile.TileContext(nc) as tc:
        with tc.tile_pool(name="const", bufs=1) as cp, \
             tc.tile_pool(name="io", bufs=4) as io, \
             tc.tile_pool(name="pr", bufs=3) as pr, \
             tc.tile_pool(name="fh", bufs=3) as fh, \
             tc.tile_pool(name="wk", bufs=2) as wk, \
             tc.tile_pool(name="ob", bufs=3) as ob, \
             tc.tile_pool(name="ps", bufs=1, space="PSUM") as ps:

            def const(d, shape, dtype=F16):
                t = cp.tile(shape, dtype, name=d.name + "_sb")
                nc.sync.dma_start(t, d.ap())
                return t

            w_wa0 = const(d_wa0, [128, 128])
            w_wa1d = const(d_wa1d, [128, 128])
            w_w2lo = const(d_w2lo, [128, 128])
            w_w2hi = const(d_w2hi, [128, 128])
            w_w2v2 = const(d_w2v2, [128, 64])
            w_wb4s = const(d_wb4s, [128, 128])
            w_wb5sd = const(d_wb5sd, [128, 128])
            w_wb4b = const(d_wb4b, [64, 128])
            w_wb5bd = const(d_wb5bd, [64, 128])
            w_wcdiag = const(d_wcdiag, [128, 128])
            w_fc0 = const(d_fc0, [128, 64])
            w_fc1 = const(d_fc1, [64, 64])
            # fc2 weights live at partitions 64:127 so the wwa/wwb matmuls
            # (moving = h2s at partitions 64:127) have matching bases
            w_fc2a_f = cp.tile([128, 128], F16, name="fc2a_sb")
            w_fc2a = w_fc2a_f[64:128, :]
            nc.sync.dma_start(w_fc2a, d_fc2a.ap())
            w_fc2bd_f = cp.tile([128, 128], F16, name="fc2bd_sb")
            w_fc2bd = w_fc2bd_f[64:128, :]
            nc.sync.dma_start(w_fc2bd, d_fc2bd.ap())
            c_b01 = const(d_b01, [128, 1], F32)
            c_b2a = const(d_b2a, [128, 1], F32)
            c_b2bh = const(d_b2bh, [128, 1], F32)
            # w3f copy at partitions 64-127 for the row-offset matmul
            w_wc_f = cp.tile([128, 64], F16, name="wc_hi_sb")
            w_wc_hi = w_wc_f[64:128, :]
            nc.sync.dma_start(w_wc_hi, d_wc2.ap())

            # per-tile state, keyed by tile index
            S = {}

            def load(k):
                t = io.tile([128, SB_COLS], F16)
                csl = slice(k * TILE_COLS, (k + 1) * TILE_COLS)
                nc.sync.dma_start(t[:, 0:TILE_COLS], d_in.ap()[:, csl])
                x2sl = d_x2c.ap()[:, k * NT:(k + 1) * NT]
                nc.sync.dma_start(
                    t[:, TILE_COLS:SB_COLS].rearrange(
                        "p (x c) -> p x c", x=4),
                    x2sl.unsqueeze(0).broadcast_to([128, 4, NT]))
                S[k] = {"T": t}

            # column offsets within the SBUF tile
            C_X1S = 0 * NT
            C_X1VA = 1 * NT
            C_X2D = 2 * NT
            C_FWT = 3 * NT
            C_VV = 4 * NT
            C_V2S = 5 * NT
            C_BS = 6 * NT
            C_BV0 = 7 * NT

            def rep(ap, n):
                return ap.unsqueeze(1).broadcast_to([128, n, NT])

            def prescale(k):
                """DVE/Pool prescales for tile k."""
                sk = S[k]
                t = sk["T"]
                x1s = t[:, C_X1S:C_X1S + NT]
                x1va = t[:, C_X1VA:C_X1VA + NT]
                # f23 = x1s * [s | v0 | v1 | v2]   [128, 4, NT]
                f23 = pr.tile([128, 4 * NT], F16)
                nc.vector.tensor_tensor(
                    f23.rearrange("p (x c) -> p x c", x=4),
                    rep(x1s, 4),
                    t[:, C_BS:C_BS + 4 * NT].rearrange(
                        "p (x c) -> p x c", x=4),
                    MULT)
                # f1a = x1va * [v0;v1]; f1b = x1va * s
                f1a = pr.tile([128, NT], F16)
                nc.vector.tensor_tensor(f1a, x1va, t[:, C_VV:C_VV + NT], MULT)
                f1b = pr.tile([128, NT], F16)
                nc.vector.tensor_tensor(f1b, x1va, t[:, C_BS:C_BS + NT], MULT)
                # xsp2 = [x1v2;x1v2] * [v2;s]   (Pool)
                xsp2 = pr.tile([128, NT], F16)
                nc.gpsimd.tensor_tensor(xsp2, t[:, C_X2D:C_X2D + NT],
                                        t[:, C_V2S:C_V2S + NT], MULT)
                sk.update(f23=f23, f1a=f1a, f1b=f1b, xsp2=xsp2,
                          x1s_s=f23[:, 0:NT], x1s_v0=f23[:, NT:2 * NT],
                          x1s_v1=f23[:, 2 * NT:3 * NT],
                          x1s_v2=f23[:, 3 * NT:4 * NT])

            def mlp_front(k, first=False, last=False):
                """h1(k+1) + h2(k) matmuls into one bank; fused silu."""
                bankH = ps.tile([128, NT], F32, tag="h12")
                if not last:          # h1 for tile k+1
                    nc.tensor.matmul(bankH[0:64, :], w_fc0,
                                     S[k + 1]["T"][:, C_FWT:C_FWT + NT],
                                     start=True, stop=True)
                if not first:         # h2 for tile k (needs h1s(k))
                    nc.tensor.matmul(bankH[64:128, :], w_fc1,
                                     S[k]["F"][0:64, :],
                                     start=True, stop=True,
                                     skip_group_check=True)
                f = fh.tile([128, NT], F16)
                if first:
                    nc.scalar.activation(f[0:64, :], bankH[0:64, :],
                                         AF.Silu, bias=c_b01[0:64, :])
                elif last:
                    nc.scalar.activation(f[64:128, :], bankH[64:128, :],
                                         AF.Silu, bias=c_b01[64:128, :])
                else:
                    nc.scalar.activation(f, bankH, AF.Silu, bias=c_b01)
                if not last:
                    S[k + 1]["F"] = f
                if not first:
                    S[k]["F2"] = f    # rows 64:128 hold h2s(k)

            def main_tp(k):
                """Six accumulating TP matmuls for tile k."""
                sk = S[k]
                scal = ps.tile([128, NT], F32, tag="scal")
                gate2 = ps.tile([128, NT], F32, tag="gate2")
                nc.tensor.matmul(scal, w_wa0, sk["x1s_s"],
                                 start=True, stop=False)
                nc.tensor.matmul(gate2, w_wa1d, sk["x1s_s"],
                                 start=True, stop=False)
                nc.tensor.matmul(scal, w_wb4s, sk["f1a"],
                                 start=False, stop=False)
                nc.tensor.matmul(gate2, w_wb5sd, sk["f1a"],
                                 start=False, stop=False)
                nc.tensor.matmul(scal, w_wb4b, sk["xsp2"][0:64, :],
                                 start=False, stop=True)
                nc.tensor.matmul(gate2, w_wb5bd, sk["xsp2"][0:64, :],
                                 start=False, stop=True)
                sk.update(scal=scal, gate2=gate2)

            def main_vec(k):
                """vec01 + vec2 into one 2-bank PSUM tile."""
                sk = S[k]
                vecall = ps.tile([128, 2 * NT], F32, tag="vecall")
                lo = vecall[:, 0:NT]
                hi = vecall[0:64, NT:2 * NT]
                nc.tensor.matmul(lo, w_wcdiag, sk["f1b"],
                                 start=True, stop=False)
                nc.tensor.matmul(lo, w_w2lo, sk["x1s_v0"],
                                 start=False, stop=False)
                nc.tensor.matmul(lo, w_w2hi, sk["x1s_v1"],
                                 start=False, stop=True)
                nc.tensor.matmul(hi, w_w2v2, sk["x1s_v2"],
                                 start=True, stop=False,
                                 skip_group_check=True)
                nc.tensor.matmul(hi, w_wc_hi, sk["xsp2"][64:128, :],
                                 start=False, stop=True,
                                 tile_position=(64, 0),
                                 skip_group_check=True)
                sk.update(vecall=vecall)

            def mlp_back(k):
                """wwa / wwb matmuls for tile k (consume h2s(k))."""
                sk = S[k]
                h2s = sk["F2"][64:128, :]
                wwa = ps.tile([128, NT], F32, tag="wwa")
                nc.tensor.matmul(wwa, w_fc2a, h2s, start=True, stop=True,
                                 tile_position=(64, 0))
                wwb = ps.tile([128, NT], F32, tag="wwb")
                nc.tensor.matmul(wwb, w_fc2bd, h2s, start=True, stop=True,
                                 tile_position=(64, 0))
                sk.update(wwa=wwa, wwb=wwb)

            def acts(k):
                sk = S[k]
                sc_silu = wk.tile([128, NT], F16)
                nc.scalar.activation(sc_silu, sk["scal"], AF.Silu)
                tg = wk.tile([128, NT], F16)
                nc.scalar.activation(tg, sk["gate2"], AF.Tanh, scale=0.5)
                wbs = wk.tile([128, NT], F16)
                nc.scalar.activation(wbs, sk["wwb"], AF.Identity, bias=c_b2bh)
                sk.update(sc_silu=sc_silu, tg=tg, wbs=wbs)

            def gatex(k):
                """tga + sigmoid-gate product, emitted right after acts(k)
                so out012(k) can fire first-thing next iteration."""
                sk = S[k]
                tga = wk.tile([128, NT], F16)
                nc.vector.tensor_scalar_add(tga, sk["tg"], 1.0)
                sgw2 = wk.tile([128, NT], F16)
                nc.gpsimd.tensor_tensor(sgw2, tga, sk["wbs"], MULT)
                sk.update(sgw2=sgw2)

            def out_phase(k):
                """Output muls + single store for tile k (PSUM frees)."""
                sk = S[k]
                O = ob.tile([128, OUT_BLK], F16)
                nc.vector.tensor_tensor(
                    O[:, NT:OUT_BLK].rearrange("p (x c) -> p x c", x=2),
                    sk["vecall"].rearrange("p (x c) -> p x c", x=2),
                    rep(sk["sgw2"], 2), MULT)
                nc.vector.scalar_tensor_tensor(
                    O[:, 0:NT], sk["wwa"], c_b2a, sk["sc_silu"], ADD, MULT)
                osl = slice(k * OUT_BLK, (k + 1) * OUT_BLK)
                nc.sync.dma_start(d_out.ap()[:, osl], O)

            # ---- pipelined emission --------------------------------
            T = T_TILES
            load(0)
            load(1)
            prescale(0)
            mlp_front(-1, first=True)       # h1(0) only -> F(0)[0:64]
            for k in range(T):
                if k + 2 < T:
                    load(k + 2)
                if k >= 1:
                    out_phase(k - 1)
                if k + 1 < T:
                    prescale(k + 1)
                mlp_front(k, last=(k + 1 >= T))   # h1(k+1), h2(k), silu
                main_tp(k)
                main_vec(k)
                mlp_back(k)
                acts(k)
                gatex(k)
                if k - 2 in S:
                    del S[k - 2]
            out_phase(T - 1)

    nc.compile()
    _CACHE["nc"] = nc
    return nc


def _fold_weights(inp):
    """Fold per-channel weights + constants into fp16 matmul matrices."""
    f = lambda k: np.asarray(inp[k], dtype=np.float32)
    w0f = f("w1_p0") * f("w2_p0")[None, :] * (INV_S * SQ2)
    w1f = f("w1_p1") * f("w2_p1")[None, :] * (INV_S * SQ2)
    w2f = f("w1_p2") * f("w2_p2")[None, :] * (INV_S * SQ2)
    w3f = f("w1_p3") * f("w2_p3")[None, :] * (INV_V * SQ2)
    w4f = f("w1_p4") * f("w2_p4")[None, :] * (INV_V * SQ3 * SQ2)
    w5f = f("w1_p5") * f("w2_p5")[None, :] * (INV_V * SQ3 * SQ2)
    fc2 = f("fc_w2")
    b2 = f("fc_b2")
    w5d = np.concatenate([w5f, w5f], axis=1)         # [64,128] col-dup
    cdiag = np.zeros((128, 128), np.float32)
    cdiag[0:64, 0:64] = w3f
    cdiag[64:128, 64:128] = w3f
    z64 = np.zeros((128, 64), np.float32)
    h = lambda a: np.ascontiguousarray(a.astype(np.float16))
    c = lambda a: np.ascontiguousarray(a.astype(np.float32))
    return {
        "wa0": h(w0f),
        "wa1d": h(np.concatenate([w1f, w1f], axis=1)),
        "w2lo": h(np.concatenate([w2f, z64], axis=1)),
        "w2hi": h(np.concatenate([z64, w2f], axis=1)),
        "w2v2": h(w2f),
        "wb4s": h(np.concatenate([w4f, w4f], axis=0)),
        "wb5sd": h(np.concatenate([w5d, w5d], axis=0)),
        "wb4b": h(w4f),
        "wb5bd": h(w5d),
        "wcdiag": h(cdiag),
        "wc2": h(w3f),
        "fc0": h(f("fc_w0")),
        "fc1": h(f("fc_w1")),
        "fc2a": h(fc2[:, :128]),
        "fc2bd": h(0.5 * np.concatenate([fc2[:, 128:], fc2[:, 128:]],
                                        axis=1)),
        "b01c": c(np.concatenate([f("fc_b0"), f("fc_b1")])[:, None]),
        "b2a": c(b2[:128, None]),
        "b2bh": c(0.5 * np.concatenate([b2[128:], b2[128:]])[:, None]),
    }


def _shard_inputs(inp):
    """Per-core merged fp16 input tensor + compact x2 rows."""
    fea_in1 = np.asarray(inp["fea_in1"], dtype=np.float32)
    fea_in2 = np.asarray(inp["fea_in2"], dtype=np.float32)
    fea_w = np.asarray(inp["fea_weight"], dtype=np.float32)
    shards = []
    for cidx in range(N_CORES):
        s = slice(cidx * E_CORE, (cidx + 1) * E_CORE)
        x1 = fea_in1[s]
        x2 = fea_in2[s]
        fw = fea_w[s]
        blocks = np.zeros((N_BLK, 128, E_PAD), np.float16)
        blocks[0][:, :E_CORE] = x1[:, :128].T.astype(np.float16)
        xv = x1[:, 128:].reshape(E_CORE, 64, 3).transpose(2, 1, 0)  # [3,64,E]
        blocks[1][0:64, :E_CORE] = xv[0].astype(np.float16)
        blocks[1][64:128, :E_CORE] = xv[1].astype(np.float16)
        blocks[2][0:64, :E_CORE] = xv[2].astype(np.float16)
        blocks[2][64:128, :E_CORE] = xv[2].astype(np.float16)
        blocks[3][:, :E_CORE] = fw.T.astype(np.float16)
        # x2 rows: [s, v0, v1, v2]
        x2p = np.zeros((4, E_PAD), np.float16)
        x2p[0, :E_CORE] = x2[:, 0].astype(np.float16)
        x2p[1, :E_CORE] = x2[:, 1].astype(np.float16)
        x2p[2, :E_CORE] = x2[:, 2].astype(np.float16)
        x2p[3, :E_CORE] = x2[:, 3].astype(np.float16)
        blocks[4][0:64, :] = x2p[1][None, :]      # vv = [v0; v1]
        blocks[4][64:128, :] = x2p[2][None, :]
        blocks[5][0:64, :] = x2p[3][None, :]      # v2s = [v2; s]
        blocks[5][64:128, :] = x2p[0][None, :]
        mega = (blocks.reshape(N_BLK, 128, T_TILES, NT)
                .transpose(1, 2, 0, 3)
                .reshape(128, T_TILES * TILE_COLS))
        shards.append({"in_t": np.ascontiguousarray(mega),
                       "x2c": np.ascontiguousarray(x2p)})
    return shards


def run(inputs, trace=False, trace_kwargs=None):
    """Run the kernel; returns (output [E,320] f32, BassKernelResults)."""
    _ensure_repo_on_path()
    from concourse import bass_utils

    nc = _build_nc()
    weights = _fold_weights(inputs)
    shards = _shard_inputs(inputs)
    in_maps = [{**weights, **sh} for sh in shards]

    kwargs = {}
    if trace:
        _install_ntff_hook()
        kwargs.update(trace=True, **(trace_kwargs or {}))
    res = bass_utils.run_bass_kernel_spmd(
        nc, in_maps, core_ids=list(range(N_CORES)), **kwargs)

    out = np.empty((E_FULL, 320), np.float32)
    for cidx in range(N_CORES):
        o = (res.results[cidx]["out_t"]
             .reshape(128, T_TILES, 3, NT).astype(np.float32))
        s = slice(cidx * E_CORE, (cidx + 1) * E_CORE)
        out_s = o[:, :, 0, :].reshape(128, E_PAD)[:, :E_CORE]
        c01 = o[:, :, 1, :].reshape(128, E_PAD)
        c2 = o[0:64, :, 2, :].reshape(64, E_PAD)
        out[s, :128] = out_s.T
        vec = np.stack([c01[0:64], c01[64:128], c2], axis=0)[:, :, :E_CORE]
        out[s, 128:] = vec.transpose(2, 1, 0).reshape(E_CORE, 192)
    return out, res


def _install_ntff_hook():
    """Shim the missing antenv.axon_hooks so trace=True works under axon."""
    import types
    import antenv
    from concourse import bass_utils
    if "antenv.axon_hooks" in sys.modules:
        return
    mod = types.ModuleType("antenv.axon_hooks")
    _h = [None]
    mod.set_axon_ntff_profile_hook = lambda h: _h.__setitem__(0, h)
    mod.get_axon_ntff_profile_hook = lambda: _h[0]
    sys.modules["antenv.axon_hooks"] = mod
    antenv.axon_hooks = mod
    from trn_agent_boot.trn_boot import _ntff_profile_via_ctypes
    mod.set_axon_ntff_profile_hook(
        _ntff_profile_via_ctypes("/opt/axon/libaxon_pjrt.so"))
    bass_utils.upload_artifacts = lambda tmpdir: tmpdir


def kernel(**inputs) -> np.ndarray:
    out, _ = run(inputs, trace=False)
    return out


# revision 10
# speedup vs baseline: 1.2695x; 1.2695x over previous
"""EquiConv (DeepH-E3) Trainium2 kernel — 8-core data-parallel over edges.

v2 strategy (channel-major, fp16, engine-rebalanced):
  - Input per tile: 6 host blocks [x1s | x1va | x2d | fwt | vv | v2s]
    (vv=[v0;v1], v2s=[v2;s] are the only host-replicated x2 rows) plus an
    on-chip broadcast DMA that fans the compact x2 rows [s|v0|v1|v2] to
    all 128 partitions of the same SBUF tile (saves ~40% input HBM).
  - ACT does 4 ops/tile: fused silu of [h1(k); h2(k-1)] packed into one
    PSUM bank, silu(scal), tanh(gate/2), and the wwb+bias crossing.
  - DVE does the big fused prescales (fp16 2x mode), tga, the
    (wwa+b2a)*silu stt, and one fused out012 multiply over a 2-bank
    PSUM tile [vec01 | vec2].
  - Pool (gpsimd) takes two SBUF-only multiplies: xsp2 prescale and the
    sigmoid-gate product sgw2.
  - One output DMA per tile: O = [out_s | out01 | out2(+64 garbage rows)]
    as [128, 1536] fp16, unpacked on host.

Self-contained: hardcodes shapes from the problem spec; no file reads.
"""
import os
import sys

import numpy as np

# ---------------------------------------------------------------- constants
E_FULL = 200000
N_CORES = 8
E_CORE = E_FULL // N_CORES      # 25000
NT = 512                        # edges per tile
T_TILES = 49                    # tiles per core
E_PAD = NT * T_TILES            # 25088
MUL_S = 128
MUL_V = 64
N_BLK = 6                       # host input row-blocks per tile
TILE_COLS = N_BLK * NT          # 3072
SB_COLS = 10 * NT               # 5120 (6 host blocks + 4 bcast blocks)
OUT_BLK = 3 * NT                # 1536

INV_S = 1.0 / np.sqrt(MUL_S)
INV_V = 1.0 / np.sqrt(MUL_V)
SQ2 = 1.0 / np.sqrt(2.0)
SQ3 = 1.0 / np.sqrt(3.0)

_REPO_CANDIDATES = (
    "/opt/trn_rl_repo",
    "/root/.axon_site/_ro/trn_rl_repo",
)


def _ensure_repo_on_path():
    try:
        import concourse.bass  # noqa: F401
        return
    except ImportError:
        pass
    for p in _REPO_CANDIDATES:
        if os.path.isdir(p) and p not in sys.path:
            sys.path.insert(0, p)
    import concourse.bass  # noqa: F401


_CACHE = {}


def _build_nc():
    """Build + compile the per-core Bass program (cached)."""
    if "nc" in _CACHE:
        return _CACHE["nc"]
    _ensure_repo_on_path()
    import concourse.mybir as mybir
    import concourse.tile as tile
    from concourse import bacc

    F32 = mybir.dt.float32
    F16 = mybir.dt.float16
    MULT = mybir.AluOpType.mult
    ADD = mybir.AluOpType.add
    AF = mybir.ActivationFunctionType

    nc = bacc.Bacc(trn_type="TRN2", target_bir_lowering=False, debug=False,
                   num_devices=N_CORES)

    # DRAM inputs (per-core shard) ----------------------------------------
    d_in = nc.dram_tensor("in_t", [128, T_TILES * TILE_COLS], F16,
                          kind="ExternalInput")
    d_x2c = nc.dram_tensor("x2c", [4, E_PAD], F16, kind="ExternalInput")
    d_wa0 = nc.dram_tensor("wa0", [128, 128], F16, kind="ExternalInput")
    d_wa1d = nc.dram_tensor("wa1d", [128, 128], F16, kind="ExternalInput")
    d_w2lo = nc.dram_tensor("w2lo", [128, 128], F16, kind="ExternalInput")
    d_w2hi = nc.dram_tensor("w2hi", [128, 128], F16, kind="ExternalInput")
    d_w2v2 = nc.dram_tensor("w2v2", [128, 64], F16, kind="ExternalInput")
    d_wb4s = nc.dram_tensor("wb4s", [128, 128], F16, kind="ExternalInput")
    d_wb5sd = nc.dram_tensor("wb5sd", [128, 128], F16, kind="ExternalInput")
    d_wb4b = nc.dram_tensor("wb4b", [64, 128], F16, kind="ExternalInput")
    d_wb5bd = nc.dram_tensor("wb5bd", [64, 128], F16, kind="ExternalInput")
    d_wcdiag = nc.dram_tensor("wcdiag", [128, 128], F16, kind="ExternalInput")
    d_wc2 = nc.dram_tensor("wc2", [64, 64], F16, kind="ExternalInput")
    d_fc0 = nc.dram_tensor("fc0", [128, 64], F16, kind="ExternalInput")
    d_fc1 = nc.dram_tensor("fc1", [64, 64], F16, kind="ExternalInput")
    d_fc2a = nc.dram_tensor("fc2a", [64, 128], F16, kind="ExternalInput")
    d_fc2bd = nc.dram_tensor("fc2bd", [64, 128], F16, kind="ExternalInput")
    d_b01 = nc.dram_tensor("b01c", [128, 1], F32, kind="ExternalInput")
    d_b2a = nc.dram_tensor("b2a", [128, 1], F32, kind="ExternalInput")
    d_b2bh = nc.dram_tensor("b2bh", [128, 1], F32, kind="ExternalInput")

    d_out = nc.dram_tensor("out_t", [128, T_TILES * OUT_BLK], F16,
                           kind="ExternalOutput")

    with tile.TileContext(nc) as tc:
        with tc.tile_pool(name="const", bufs=1) as cp, \
             tc.tile_pool(name="io", bufs=4) as io, \
             tc.tile_pool(name="pr", bufs=3) as pr, \
             tc.tile_pool(name="fh", bufs=3) as fh, \
             tc.tile_pool(name="wk", bufs=2) as wk, \
             tc.tile_pool(name="ob", bufs=3) as ob, \
             tc.tile_pool(name="ps", bufs=1, space="PSUM") as ps:

            def const(d, shape, dtype=F16):
                t = cp.tile(shape, dtype, name=d.name + "_sb")
                nc.sync.dma_start(t, d.ap())
                return t

            w_wa0 = const(d_wa0, [128, 128])
            w_wa1d = const(d_wa1d, [128, 128])
            w_w2lo = const(d_w2lo, [128, 128])
            w_w2hi = const(d_w2hi, [128, 128])
            w_w2v2 = const(d_w2v2, [128, 64])
            w_wb4s = const(d_wb4s, [128, 128])
            w_wb5sd = const(d_wb5sd, [128, 128])
            w_wb4b = const(d_wb4b, [64, 128])
            w_wb5bd = const(d_wb5bd, [64, 128])
            w_wcdiag = const(d_wcdiag, [128, 128])
            w_fc0 = const(d_fc0, [128, 64])
            w_fc1 = const(d_fc1, [64, 64])
            # fc2 weights live at partitions 64:127 so the wwa/wwb matmuls
            # (moving = h2s at partitions 64:127) have matching bases
            w_fc2a_f = cp.tile([128, 128], F16, name="fc2a_sb")
            w_fc2a = w_fc2a_f[64:128, :]
            nc.sync.dma_start(w_fc2a, d_fc2a.ap())
            w_fc2bd_f = cp.tile([128, 128], F16, name="fc2bd_sb")
            w_fc2bd = w_fc2bd_f[64:128, :]
            nc.sync.dma_start(w_fc2bd, d_fc2bd.ap())
            c_b01 = const(d_b01, [128, 1], F32)
            c_b2a = const(d_b2a, [128, 1], F32)
            c_b2bh = const(d_b2bh, [128, 1], F32)
            # w3f copy at partitions 64-127 for the row-offset matmul
            w_wc_f = cp.tile([128, 64], F16, name="wc_hi_sb")
            w_wc_hi = w_wc_f[64:128, :]
            nc.sync.dma_start(w_wc_hi, d_wc2.ap())

            # per-tile state, keyed by tile index
            S = {}

            def load(k):
                t = io.tile([128, SB_COLS], F16)
                csl = slice(k * TILE_COLS, (k + 1) * TILE_COLS)
                nc.sync.dma_start(t[:, 0:TILE_COLS], d_in.ap()[:, csl])
                x2sl = d_x2c.ap()[:, k * NT:(k + 1) * NT]
                nc.sync.dma_start(
                    t[:, TILE_COLS:SB_COLS].rearrange(
                        "p (x c) -> p x c", x=4),
                    x2sl.unsqueeze(0).broadcast_to([128, 4, NT]))
                S[k] = {"T": t}

            # column offsets within the SBUF tile
            C_X1S = 0 * NT
            C_X1VA = 1 * NT
            C_X2D = 2 * NT
            C_FWT = 3 * NT
            C_VV = 4 * NT
            C_V2S = 5 * NT
            C_BS = 6 * NT
            C_BV0 = 7 * NT

            def rep(ap, n):
                return ap.unsqueeze(1).broadcast_to([128, n, NT])

            def prescale(k):
                """DVE/Pool prescales for tile k."""
                sk = S[k]
                t = sk["T"]
                x1s = t[:, C_X1S:C_X1S + NT]
                x1va = t[:, C_X1VA:C_X1VA + NT]
                # f23 = x1s * [s | v0 | v1 | v2]   [128, 4, NT]
                f23 = pr.tile([128, 4 * NT], F16)
                nc.vector.tensor_tensor(
                    f23.rearrange("p (x c) -> p x c", x=4),
                    rep(x1s, 4),
                    t[:, C_BS:C_BS + 4 * NT].rearrange(
                        "p (x c) -> p x c", x=4),
                    MULT)
                # f1a = x1va * [v0;v1]; f1b = x1va * s   (Pool)
                f1a = pr.tile([128, NT], F16)
                nc.gpsimd.tensor_tensor(f1a, x1va, t[:, C_VV:C_VV + NT], MULT)
                f1b = pr.tile([128, NT], F16)
                nc.gpsimd.tensor_tensor(f1b, x1va, t[:, C_BS:C_BS + NT], MULT)
                # xsp2 = [x1v2;x1v2] * [v2;s]   (Pool)
                xsp2 = pr.tile([128, NT], F16)
                nc.gpsimd.tensor_tensor(xsp2, t[:, C_X2D:C_X2D + NT],
                                        t[:, C_V2S:C_V2S + NT], MULT)
                sk.update(f23=f23, f1a=f1a, f1b=f1b, xsp2=xsp2,
                          x1s_s=f23[:, 0:NT], x1s_v0=f23[:, NT:2 * NT],
                          x1s_v1=f23[:, 2 * NT:3 * NT],
                          x1s_v2=f23[:, 3 * NT:4 * NT])

            def mlp_front(k, first=False, last=False):
                """h1(k+1) + h2(k) matmuls into one bank; fused silu."""
                bankH = ps.tile([128, NT], F32, tag="h12")
                if not last:          # h1 for tile k+1
                    nc.tensor.matmul(bankH[0:64, :], w_fc0,
                                     S[k + 1]["T"][:, C_FWT:C_FWT + NT],
                                     start=True, stop=True)
                if not first:         # h2 for tile k (needs h1s(k))
                    nc.tensor.matmul(bankH[64:128, :], w_fc1,
                                     S[k]["F"][0:64, :],
                                     start=True, stop=True,
                                     skip_group_check=True)
                f = fh.tile([128, NT], F16)
                if first:
                    nc.scalar.activation(f[0:64, :], bankH[0:64, :],
                                         AF.Silu, bias=c_b01[0:64, :])
                elif last:
                    nc.scalar.activation(f[64:128, :], bankH[64:128, :],
                                         AF.Silu, bias=c_b01[64:128, :])
                else:
                    nc.scalar.activation(f, bankH, AF.Silu, bias=c_b01)
                if not last:
                    S[k + 1]["F"] = f
                if not first:
                    S[k]["F2"] = f    # rows 64:128 hold h2s(k)

            def main_tp(k):
                """Six accumulating TP matmuls for tile k."""
                sk = S[k]
                scal = ps.tile([128, NT], F32, tag="scal")
                gate2 = ps.tile([128, NT], F32, tag="gate2")
                nc.tensor.matmul(scal, w_wa0, sk["x1s_s"],
                                 start=True, stop=False)
                nc.tensor.matmul(gate2, w_wa1d, sk["x1s_s"],
                                 start=True, stop=False)
                nc.tensor.matmul(scal, w_wb4s, sk["f1a"],
                                 start=False, stop=False)
                nc.tensor.matmul(gate2, w_wb5sd, sk["f1a"],
                                 start=False, stop=False)
                nc.tensor.matmul(scal, w_wb4b, sk["xsp2"][0:64, :],
                                 start=False, stop=True)
                nc.tensor.matmul(gate2, w_wb5bd, sk["xsp2"][0:64, :],
                                 start=False, stop=True)
                sk.update(scal=scal, gate2=gate2)

            def main_vec(k):
                """vec01 + vec2 into one 2-bank PSUM tile."""
                sk = S[k]
                vecall = ps.tile([128, 2 * NT], F32, tag="vecall")
                lo = vecall[:, 0:NT]
                hi = vecall[0:64, NT:2 * NT]
                nc.tensor.matmul(lo, w_wcdiag, sk["f1b"],
                                 start=True, stop=False)
                nc.tensor.matmul(lo, w_w2lo, sk["x1s_v0"],
                                 start=False, stop=False)
                nc.tensor.matmul(lo, w_w2hi, sk["x1s_v1"],
                                 start=False, stop=True)
                nc.tensor.matmul(hi, w_w2v2, sk["x1s_v2"],
                                 start=True, stop=False,
                                 skip_group_check=True)
                nc.tensor.matmul(hi, w_wc_hi, sk["xsp2"][64:128, :],
                                 start=False, stop=True,
                                 tile_position=(64, 0),
                                 skip_group_check=True)
                sk.update(vecall=vecall)

            def mlp_back(k):
                """wwa / wwb matmuls for tile k (consume h2s(k))."""
                sk = S[k]
                h2s = sk["F2"][64:128, :]
                wwa = ps.tile([128, NT], F32, tag="wwa")
                nc.tensor.matmul(wwa, w_fc2a, h2s, start=True, stop=True,
                                 tile_position=(64, 0))
                wwb = ps.tile([128, NT], F32, tag="wwb")
                nc.tensor.matmul(wwb, w_fc2bd, h2s, start=True, stop=True,
                                 tile_position=(64, 0))
                sk.update(wwa=wwa, wwb=wwb)

            def acts(k):
                sk = S[k]
                sc_silu = wk.tile([128, NT], F16)
                nc.scalar.activation(sc_silu, sk["scal"], AF.Silu)
                tg = wk.tile([128, NT], F16)
                nc.scalar.activation(tg, sk["gate2"], AF.Tanh, scale=0.5)
                wbs = wk.tile([128, NT], F16)
                nc.scalar.activation(wbs, sk["wwb"], AF.Identity, bias=c_b2bh)
                sk.update(sc_silu=sc_silu, tg=tg, wbs=wbs)

            def gatex(k):
                """sgw2 = (tg+1)*wbs in one DVE stt, emitted after acts(k)
                so out012(k) can fire first-thing next iteration."""
                sk = S[k]
                sgw2 = wk.tile([128, NT], F16)
                nc.vector.scalar_tensor_tensor(
                    sgw2, sk["tg"], 1.0, sk["wbs"], ADD, MULT)
                sk.update(sgw2=sgw2)

            def out_phase(k):
                """Output muls + single store for tile k (PSUM frees)."""
                sk = S[k]
                O = ob.tile([128, OUT_BLK], F16)
                nc.vector.tensor_tensor(
                    O[:, NT:OUT_BLK].rearrange("p (x c) -> p x c", x=2),
                    sk["vecall"].rearrange("p (x c) -> p x c", x=2),
                    rep(sk["sgw2"], 2), MULT)
                nc.vector.scalar_tensor_tensor(
                    O[:, 0:NT], sk["wwa"], c_b2a, sk["sc_silu"], ADD, MULT)
                osl = slice(k * OUT_BLK, (k + 1) * OUT_BLK)
                nc.sync.dma_start(d_out.ap()[:, osl], O)

            # ---- pipelined emission --------------------------------
            T = T_TILES
            load(0)
            load(1)
            prescale(0)
            mlp_front(-1, first=True)       # h1(0) only -> F(0)[0:64]
            for k in range(T):
                if k + 2 < T:
                    load(k + 2)
                if k >= 1:
                    out_phase(k - 1)
                if k + 1 < T:
                    prescale(k + 1)
                mlp_front(k, last=(k + 1 >= T))   # h1(k+1), h2(k), silu
                main_tp(k)
                main_vec(k)
                mlp_back(k)
                acts(k)
                gatex(k)
                if k - 2 in S:
                    del S[k - 2]
            out_phase(T - 1)

    nc.compile()
    _CACHE["nc"] = nc
    return nc


def _fold_weights(inp):
    """Fold per-channel weights + constants into fp16 matmul matrices."""
    f = lambda k: np.asarray(inp[k], dtype=np.float32)
    w0f = f("w1_p0") * f("w2_p0")[None, :] * (INV_S * SQ2)
    w1f = f("w1_p1") * f("w2_p1")[None, :] * (INV_S * SQ2)
    w2f = f("w1_p2") * f("w2_p2")[None, :] * (INV_S * SQ2)
    w3f = f("w1_p3") * f("w2_p3")[None, :] * (INV_V * SQ2)
    w4f = f("w1_p4") * f("w2_p4")[None, :] * (INV_V * SQ3 * SQ2)
    w5f = f("w1_p5") * f("w2_p5")[None, :] * (INV_V * SQ3 * SQ2)
    fc2 = f("fc_w2")
    b2 = f("fc_b2")
    w5d = np.concatenate([w5f, w5f], axis=1)         # [64,128] col-dup
    cdiag = np.zeros((128, 128), np.float32)
    cdiag[0:64, 0:64] = w3f
    cdiag[64:128, 64:128] = w3f
    z64 = np.zeros((128, 64), np.float32)
    h = lambda a: np.ascontiguousarray(a.astype(np.float16))
    c = lambda a: np.ascontiguousarray(a.astype(np.float32))
    return {
        "wa0": h(w0f),
        "wa1d": h(np.concatenate([w1f, w1f], axis=1)),
        "w2lo": h(np.concatenate([w2f, z64], axis=1)),
        "w2hi": h(np.concatenate([z64, w2f], axis=1)),
        "w2v2": h(w2f),
        "wb4s": h(np.concatenate([w4f, w4f], axis=0)),
        "wb5sd": h(np.concatenate([w5d, w5d], axis=0)),
        "wb4b": h(w4f),
        "wb5bd": h(w5d),
        "wcdiag": h(cdiag),
        "wc2": h(w3f),
        "fc0": h(f("fc_w0")),
        "fc1": h(f("fc_w1")),
        "fc2a": h(fc2[:, :128]),
        "fc2bd": h(0.5 * np.concatenate([fc2[:, 128:], fc2[:, 128:]],
                                        axis=1)),
        "b01c": c(np.concatenate([f("fc_b0"), f("fc_b1")])[:, None]),
        "b2a": c(b2[:128, None]),
        "b2bh": c(0.5 * np.concatenate([b2[128:], b2[128:]])[:, None]),
    }


def _shard_inputs(inp):
    """Per-core merged fp16 input tensor + compact x2 rows."""
    fea_in1 = np.asarray(inp["fea_in1"], dtype=np.float32)
    fea_in2 = np.asarray(inp["fea_in2"], dtype=np.float32)
    fea_w = np.asarray(inp["fea_weight"], dtype=np.float32)
    shards = []
    for cidx in range(N_CORES):
        s = slice(cidx * E_CORE, (cidx + 1) * E_CORE)
        x1 = fea_in1[s]
        x2 = fea_in2[s]
        fw = fea_w[s]
        blocks = np.zeros((N_BLK, 128, E_PAD), np.float16)
        blocks[0][:, :E_CORE] = x1[:, :128].T.astype(np.float16)
        xv = x1[:, 128:].reshape(E_CORE, 64, 3).transpose(2, 1, 0)  # [3,64,E]
        blocks[1][0:64, :E_CORE] = xv[0].astype(np.float16)
        blocks[1][64:128, :E_CORE] = xv[1].astype(np.float16)
        blocks[2][0:64, :E_CORE] = xv[2].astype(np.float16)
        blocks[2][64:128, :E_CORE] = xv[2].astype(np.float16)
        blocks[3][:, :E_CORE] = fw.T.astype(np.float16)
        # x2 rows: [s, v0, v1, v2]
        x2p = np.zeros((4, E_PAD), np.float16)
        x2p[0, :E_CORE] = x2[:, 0].astype(np.float16)
        x2p[1, :E_CORE] = x2[:, 1].astype(np.float16)
        x2p[2, :E_CORE] = x2[:, 2].astype(np.float16)
        x2p[3, :E_CORE] = x2[:, 3].astype(np.float16)
        blocks[4][0:64, :] = x2p[1][None, :]      # vv = [v0; v1]
        blocks[4][64:128, :] = x2p[2][None, :]
        blocks[5][0:64, :] = x2p[3][None, :]      # v2s = [v2; s]
        blocks[5][64:128, :] = x2p[0][None, :]
        mega = (blocks.reshape(N_BLK, 128, T_TILES, NT)
                .transpose(1, 2, 0, 3)
                .reshape(128, T_TILES * TILE_COLS))
        shards.append({"in_t": np.ascontiguousarray(mega),
                       "x2c": np.ascontiguousarray(x2p)})
    return shards


def run(inputs, trace=False, trace_kwargs=None):
    """Run the kernel; returns (output [E,320] f32, BassKernelResults)."""
    _ensure_repo_on_path()
    from concourse import bass_utils

    nc = _build_nc()
    weights = _fold_weights(inputs)
    shards = _shard_inputs(inputs)
    in_maps = [{**weights, **sh} for sh in shards]

    kwargs = {}
    if trace:
        _install_ntff_hook()
        kwargs.update(trace=True, **(trace_kwargs or {}))
    res = bass_utils.run_bass_kernel_spmd(
        nc, in_maps, core_ids=list(range(N_CORES)), **kwargs)

    out = np.empty((E_FULL, 320), np.float32)
    for cidx in range(N_CORES):
        o = (res.results[cidx]["out_t"]
             .reshape(128, T_TILES, 3, NT).astype(np.float32))
        s = slice(cidx * E_CORE, (cidx + 1) * E_CORE)
        out_s = o[:, :, 0, :].reshape(128, E_PAD)[:, :E_CORE]
        c01 = o[:, :, 1, :].reshape(128, E_PAD)
        c2 = o[0:64, :, 2, :].reshape(64, E_PAD)
        out[s, :128] = out_s.T
        vec = np.stack([c01[0:64], c01[64:128], c2], axis=0)[:, :, :E_CORE]
        out[s, 128:] = vec.transpose(2, 1, 0).reshape(E_CORE, 192)
    return out, res


def _install_ntff_hook():
    """Shim the missing antenv.axon_hooks so trace=True works under axon."""
    import types
    import antenv
    from concourse import bass_utils
    if "antenv.axon_hooks" in sys.modules:
        return
    mod = types.ModuleType("antenv.axon_hooks")
    _h = [None]
    mod.set_axon_ntff_profile_hook = lambda h: _h.__setitem__(0, h)
    mod.get_axon_ntff_profile_hook = lambda: _h[0]
    sys.modules["antenv.axon_hooks"] = mod
    antenv.axon_hooks = mod
    from trn_agent_boot.trn_boot import _ntff_profile_via_ctypes
    mod.set_axon_ntff_profile_hook(
        _ntff_profile_via_ctypes("/opt/axon/libaxon_pjrt.so"))
    bass_utils.upload_artifacts = lambda tmpdir: tmpdir


def kernel(**inputs) -> np.ndarray:
    out, _ = run(inputs, trace=False)
    return out
